# revision 41
# baseline (speedup 1.0000x reference)
"""Trainium2 Bass kernel for DecomposingAttnProcessor (pad variant).

Math (pad branch contributes exactly zero since pad tokens are zeros
projected with no bias -> k_pad = v_pad = 0):
    q = hs @ Wq.T / (temp + eps)   (scale folded into Wq on host)
    k = ehs @ Wk.T ; v = ehs @ Wv.T
    scores[c,h,s,e] = q . k        (per head, dh=64)
    w = softmax over the 4 components c (dim 0)
    o = w @ v ; out = o @ Wo.T + bo + hs

Sharding: 8 cores, split S=4096 into 512-row blocks; all 4 components of
a block stay on one core (softmax couples them). K/V computed redundantly
per core (encoder seq is only 154).

All matmuls run at N=512 (one full PSUM bank) to amortize the per-matmul
LDWEIGHTS/dispatch tax; the whole 512-row s-block is processed in one
pass (no s-halves).

Encoder layout (host-packed, 640 columns):
    cols [c*128,(c+1)*128) = component c, e in [0,128)   ("main")
    cols [512+c*32, 512+c*32+32) = component c, e in [128,154) zero-padded
    to 32 ("tail") so every matmul operand is 32-aligned on partitions.

Concurrency: small matmuls (tails, score halves) are issued back-to-back
to disjoint 32-strip PE tiles (tile_position) so they run concurrently
instead of serially; the tail softmax sum+broadcast is a single constant
matmul (Msum) instead of two.

DMA: eT/wv/xh/wq/wo stream on the sync queue; wk blocks + wq00 + small
constants go on the otherwise-idle gpsimd queue so the two queues issue
descriptors in parallel at kernel start.  Outputs (bf16) are written from
the scalar queue, which is idle during the out-projection phase.
"""

import numpy as np
import ml_dtypes

import concourse.bass as bass
import concourse.mybir as mybir
import concourse.tile as tile
from concourse import bacc
from concourse.bass_utils import run_bass_kernel_spmd

F32 = mybir.dt.float32
BF16 = mybir.dt.bfloat16
AF = mybir.ActivationFunctionType
ALU = mybir.AluOpType

NCOMP = 4
HEADS = 24
DH = 64
D = 1536
S = 4096
E = 154
EM = 128                  # main e-rows per component
ET = E - EM               # 26 tail e-rows per component
EPS = 1e-8
NCORES = 8
SL = S // NCORES          # 512 s-rows per core (per component)
FT = D // 128             # 12 feature tiles of 128
HP = HEADS // 2           # 12 head-pairs (2 heads = 128 feature rows)
ECAT = 640                # 4*128 main + 4*32 padded tail columns
TB = 4 * EM               # 512: tail block column base


def _emit(tc):
    import os
    phases = os.environ.get("K_PHASES", "ABC")
    blevel = int(os.environ.get("K_BLEVEL", "4"))
    nc = tc.nc

    xTb = nc.declare_dram_parameter("xTb", [NCOMP, D, SL], BF16, isOutput=False)
    eT = nc.declare_dram_parameter("eT", [D, ECAT], BF16, isOutput=False)
    wqT = nc.declare_dram_parameter("wqT", [D, D], BF16, isOutput=False)
    # wk is host-packed partition-major: each block loads as one flat
    # 2-level descriptor (~1.6us) instead of a 3-level pattern (~3.5us),
    # keeping the K chains (one block per 3.2us) fed at startup
    wkT = nc.declare_dram_parameter("wkT", [FT, 128, FT * 128], BF16,
                                    isOutput=False)
    wvT = nc.declare_dram_parameter("wvT", [D, D], BF16, isOutput=False)
    woT = nc.declare_dram_parameter("woT", [D, D], BF16, isOutput=False)
    bo = nc.declare_dram_parameter("bo", [128, FT], F32, isOutput=False)
    msum = nc.declare_dram_parameter("msum", [128, 128], BF16, isOutput=False)
    outT = nc.declare_dram_parameter("outT", [NCOMP, D, SL], BF16, isOutput=True)

    xTb_v = [xTb[c].rearrange("(f p) s -> p f s", p=128) for c in range(NCOMP)]
    eT_v = eT.rearrange("(f p) e -> p f e", p=128)
    wqT_v = wqT.rearrange("(f p) o -> p f o", p=128)
    wvT_v = wvT.rearrange("(f p) o -> p f o", p=128)
    woT_v = woT.rearrange("(f p) o -> p f o", p=128)
    outT_v = [outT[c].rearrange("(f p) s -> p f s", p=128) for c in range(NCOMP)]

    with tc.tile_pool(name="persist", bufs=1) as pp:
        # ---------------- persistent tiles ----------------
        kt_sb = [pp.tile([128, ECAT], BF16, tag="kT", bufs=FT, name=f"kt{t}")
                 for t in range(FT)]
        vm_sb = [pp.tile([128, D], BF16, tag="vm", bufs=NCOMP, name=f"vm{c}")
                 for c in range(NCOMP)]
        vt_sb = pp.tile([128, D], BF16, tag="vt", bufs=1, name="vt")
        bo_sb = pp.tile([128, FT], F32, tag="bo", bufs=1, name="bo_sb")
        msum_sb = pp.tile([128, 128], BF16, tag="msum", bufs=1, name="msum_sb")

        # x panels (Q rhs + residual source) and the first wq block; bulk
        # loads go on the sync queue behind eT/wv in priority order
        xh = [pp.tile([128, FT * SL], BF16, tag="xh", bufs=NCOMP,
                      name=f"xh_{c}") for c in range(NCOMP)]
        wq00 = pp.tile([128, FT * 128], BF16, tag="wq00", bufs=1, name="wq00")

        def _load_xh(c):
            nc.sync.dma_start(
                out=xh[c].rearrange("p (f s) -> p f s", f=FT), in_=xTb_v[c])

        def _load_wq00():
            # gpsimd queue: parallel with the sync-queue bulk stream
            nc.gpsimd.dma_start(
                out=wq00.rearrange("p (f o) -> p f o", f=FT),
                in_=wqT_v[:, :, 0:128])

        def _phases():
            # ---------------- phase A: K^T and V ----------------
            if "A" in phases:
              with (
                tc.tile_pool(name="pha", bufs=1) as pa,
                tc.tile_pool(name="pha_psum", bufs=1, space="PSUM") as pap,
              ):
                # wk0 then eT in 4 chunks of 3 feature-tiles on the sync
                # queue: few descriptors (the sync queue issues one per
                # ~650ns) but enough that they spread over parallel DMA
                # engines instead of crawling through one
                wk_all = []
                wk0 = pa.tile([128, FT * 128], BF16, tag="wk",
                              bufs=4, name="wk0")
                nc.sync.dma_start(out=wk0[:], in_=wkT[0])
                wk_all.append(wk0)
                # per-fi 2-level descriptors: a single flat [128, 1280B]
                # transfer sustains ~250 GB/s while 3-level patterns crawl
                # at ~90 GB/s; the second half goes out on the (idle)
                # scalar queue so the two halves stream in parallel
                et_b = pa.tile([128, FT * ECAT], BF16, tag="eT", bufs=1,
                               name="et_b")
                for fi in range(FT // 2):
                    nc.sync.dma_start(
                        out=et_b[:, fi * ECAT:(fi + 1) * ECAT],
                        in_=eT_v[:, fi])
                for fi in range(FT // 2, FT):
                    nc.scalar.dma_start(
                        out=et_b[:, fi * ECAT:(fi + 1) * ECAT],
                        in_=eT_v[:, fi])
                et = [et_b[:, fi * ECAT:(fi + 1) * ECAT] for fi in range(FT)]

                # remaining wk blocks alternate gpsimd (odd) / sync (even)
                # so neither queue falls behind the chains' consumption
                for fot in range(1, FT):
                    wk_b = pa.tile([128, FT * 128], BF16, tag="wk",
                                   bufs=4, name=f"wk{fot}")
                    if fot % 2 == 1:
                        nc.gpsimd.dma_start(out=wk_b[:], in_=wkT[fot])
                    else:
                        nc.sync.dma_start(out=wk_b[:], in_=wkT[fot])
                    wk_all.append(wk_b)
                    if fot == 5:
                        nc.gpsimd.dma_start(out=bo_sb[:], in_=bo[:])
                        nc.gpsimd.dma_start(out=msum_sb[:], in_=msum[:])
                    if fot == 9:
                        _load_wq00()

                # wv + xh stream on sync behind eT, two descriptors each
                wv_all = []
                for fvc in range(3):
                    wv_b = pa.tile([128, FT * 512], BF16, tag="wv", bufs=3,
                                   name=f"wv{fvc}")
                    wv_v3 = wv_b.rearrange("p (f o) -> p f o", f=FT)
                    for ch in range(2):
                        nc.sync.dma_start(
                            out=wv_v3[:, 6 * ch:6 * ch + 6],
                            in_=wvT_v[:, 6 * ch:6 * ch + 6,
                                      fvc * 512:(fvc + 1) * 512])
                    wv_all.append(wv_b)
                for c in range(NCOMP):
                    xh_v3 = xh[c].rearrange("p (f s) -> p f s", f=FT)
                    for ch in range(2):
                        nc.sync.dma_start(
                            out=xh_v3[:, 6 * ch:6 * ch + 6],
                            in_=xTb_v[c][:, 6 * ch:6 * ch + 6])

                # K^T[fo, col] over fi; N split 320+320
                for fot in range(FT):
                    wk_b = wk_all[fot]
                    for nch in range(2):
                        n0 = nch * 320
                        pk = pap.tile([128, 320], F32, tag="pk", bufs=2,
                                      name=f"pk{fot}_{nch}")
                        for fi in range(FT):
                            nc.tensor.matmul(
                                pk[:], wk_b[:, fi * 128:(fi + 1) * 128],
                                et[fi][:, n0:n0 + 320],
                                start=(fi == 0), stop=(fi == FT - 1))
                        nc.scalar.copy(kt_sb[fot][:, n0:n0 + 320], pk[:])

                # V (natural layout [e, dv], bf16) over fi; tails of all 4
                # components go concurrently to disjoint column groups
                for fvc in range(3):
                    wv_b = wv_all[fvc]
                    for c in range(NCOMP):
                        pv = pap.tile([128, 512], F32, tag="pv", bufs=2,
                                      name=f"pv{fvc}_{c}")
                        for fi in range(FT):
                            nc.tensor.matmul(
                                pv[:],
                                et[fi][:, c * EM:(c + 1) * EM],
                                wv_b[:, fi * 512:(fi + 1) * 512],
                                start=(fi == 0), stop=(fi == FT - 1))
                        nc.scalar.copy(
                            vm_sb[c][:, fvc * 512:(fvc + 1) * 512], pv[:])
                    pvt = pap.tile([128, 512], F32, tag="pv", bufs=2,
                                   name=f"pvt{fvc}")
                    for fi in range(FT):
                        for c in range(NCOMP):
                            nc.tensor.matmul(
                                pvt[c * 32:(c + 1) * 32, :],
                                et[fi][:, TB + c * 32:TB + (c + 1) * 32],
                                wv_b[:, fi * 512:(fi + 1) * 512],
                                start=(fi == 0), stop=(fi == FT - 1),
                                skip_group_check=True,
                                tile_position=(0, c * 32))
                    nc.scalar.copy(
                        vt_sb[:, fvc * 512:(fvc + 1) * 512], pvt[:])
            if "A" not in phases:
                nc.gpsimd.dma_start(out=bo_sb[:], in_=bo[:])
                nc.gpsimd.dma_start(out=msum_sb[:], in_=msum[:])
                for c in range(NCOMP):
                    _load_xh(c)
                _load_wq00()

            # ---------------- phase B: Q, scores, softmax, o ----------------
            with tc.tile_pool(name="bc", bufs=1) as bc:
              with tc.tile_pool(name="bcp", bufs=1, space="PSUM") as bcp:
                ot_sb = {}
                pend = None     # (hp, w_big, ex_t, rinvt) awaiting mul+AV

                # pre-issue the first wo loads so phase C starts without a
                # DMA wait (ring slots recycle for fot >= 3)
                wo_pre = []
                for fot in range(3 if "C" in phases else 0):
                    wo_b = bc.tile([128, FT * 128], BF16, tag="wo", bufs=3,
                                   name=f"wo{fot}")
                    nc.sync.dma_start(
                        out=wo_b.rearrange("p (f o) -> p f o", f=FT),
                        in_=woT_v[:, :, fot * 128:(fot + 1) * 128])
                    wo_pre.append(wo_b)

                def _tail_sum(hp, ex_t):
                    # one constant matmul sums the 4 component groups AND
                    # re-broadcasts the sum to every group's partitions:
                    # tps[c*32+j, s] = sum_c' ex_t[c'*32+j, s]
                    rinvt = bc.tile([128, 2 * SL], F32, tag="rinvt", bufs=2,
                                    name=f"rinvt{hp}")
                    for sh in range(2):
                        tps = bcp.tile([128, 512], F32, tag="ps", bufs=3,
                                       name=f"tps{hp}_{sh}")
                        nc.tensor.matmul(tps[:], msum_sb[:],
                                         ex_t[:, sh * 512:(sh + 1) * 512],
                                         start=True, stop=True)
                        nc.vector.reciprocal_approx_fast(
                            out=rinvt[:, sh * 512:(sh + 1) * 512],
                            in_=tps[:])
                    return rinvt

                def _tail_mul_av(hp, w_big, ex_t, rinvt):
                    # normalize the tail in place (on the otherwise-idle
                    # gpsimd engine so the DVE stays off this critical
                    # path), then AV: mains first (col-paired), then all 8
                    # tails concurrently into disjoint 32-strip PE tiles
                    # accumulating on the mains
                    nc.vector.tensor_mul(out=ex_t[:], in0=ex_t[:],
                                         in1=rinvt[:])
                    w_t = ex_t
                    for cp in range(2 if blevel >= 4 else 0):
                        po = {}
                        for c in (2 * cp, 2 * cp + 1):
                            po[c] = bcp.tile([128, SL], F32, tag="po", bufs=3,
                                             name=f"po{hp}_{c}")
                        for c in (2 * cp, 2 * cp + 1):
                            h0 = hp * 2
                            for hh in range(2):
                                nc.tensor.matmul(
                                    po[c][hh * 64:(hh + 1) * 64, :],
                                    vm_sb[c][:, (h0 + hh) * 64:
                                             (h0 + hh + 1) * 64],
                                    w_big[:, c, hh * SL:(hh + 1) * SL],
                                    start=True, stop=False,
                                    skip_group_check=True,
                                    tile_position=(0, hh * 64))
                        for c in (2 * cp, 2 * cp + 1):
                            h0 = hp * 2
                            for hh in range(2):
                                nc.tensor.matmul(
                                    po[c][hh * 64:(hh + 1) * 64, :],
                                    vt_sb[c * 32:c * 32 + ET,
                                          (h0 + hh) * 64:(h0 + hh + 1) * 64],
                                    w_t[c * 32:c * 32 + ET,
                                        hh * SL:(hh + 1) * SL],
                                    start=False, stop=True,
                                    skip_group_check=True,
                                    tile_position=(c * 32, hh * 64))
                        for c in (2 * cp, 2 * cp + 1):
                            ot = bc.tile([128, SL], BF16, tag="oT", bufs=48,
                                         name=f"ot{hp}_{c}")
                            # scalar engine: the DVE queue is busier here
                            nc.scalar.copy(ot[:], po[c][:])
                            ot_sb[(c, hp)] = ot

                for hp in range(HP if "B" in phases else 0):
                    if hp == 0:
                        wq_b = wq00
                    else:
                        wq_b = bc.tile([128, FT * 128], BF16, tag="wq",
                                       bufs=3, name=f"wq{hp}")
                        nc.sync.dma_start(
                            out=wq_b.rearrange("p (f o) -> p f o", f=FT),
                            in_=wqT_v[:, :, hp * 128:(hp + 1) * 128])

                    # Q^T chains, interleaved with the previous hp's
                    # deferred tail-softmax ops so the PE never waits
                    # on the scalar/vector engines
                    qt = []
                    rinvt_prev = None
                    for c in range(NCOMP):
                        pq = bcp.tile([128, SL], F32, tag="pq", bufs=2,
                                      name=f"pq{hp}_{c}")
                        for fi in range(FT):
                            nc.tensor.matmul(
                                pq[:], wq_b[:, fi * 128:(fi + 1) * 128],
                                xh[c][:, fi * SL:(fi + 1) * SL],
                                start=(fi == 0), stop=(fi == FT - 1))
                        q = bc.tile([128, SL], BF16, tag="qT", bufs=8,
                                    name=f"qt{hp}_{c}")
                        nc.vector.tensor_copy(out=q[:], in_=pq[:])
                        qt.append(q)
                        if c == 1 and pend is not None:
                            rinvt_prev = _tail_sum(pend[0], pend[2])
                    if pend is not None:
                        _tail_mul_av(pend[0], pend[1], pend[2], rinvt_prev)
                        pend = None
                    if blevel < 2:
                        continue

                    # scores + exp; main part: c along the free dim of
                    # one big tile; the two head-halves of each component
                    # go back-to-back into disjoint row strips (concurrent)
                    ex_big = bc.tile([128, NCOMP, 2 * SL], BF16, tag="exb",
                                     bufs=2, name=f"exb{hp}")
                    for c in range(NCOMP):
                        ps = {}
                        for hh in range(2):
                            ps[hh] = bcp.tile([128, SL], F32, tag="ps",
                                              bufs=3, name=f"ps{hp}_{c}_{hh}")
                            nc.tensor.matmul(
                                ps[hh][:],
                                kt_sb[hp][hh * 64:(hh + 1) * 64,
                                          c * EM:(c + 1) * EM],
                                qt[c][hh * 64:(hh + 1) * 64, :],
                                start=True, stop=True,
                                skip_group_check=True,
                                tile_position=(hh * 64, 0))
                        for hh in range(2):
                            # heads packed side by side along the free dim
                            nc.scalar.activation(
                                ex_big[:, c, hh * 512:(hh + 1) * 512],
                                ps[hh][:], AF.Exp)
                    # tail scores: all 8 (hh x c) tiles disjoint -> one
                    # concurrent burst across two PSUM banks
                    ex_t = bc.tile([128, 2 * SL], BF16, tag="ext",
                                   bufs=2, name=f"ext{hp}")
                    pst = {}
                    for hh in range(2):
                        pst[hh] = bcp.tile([128, SL], F32, tag="ps", bufs=3,
                                           name=f"pst{hp}_{hh}")
                    for hh in range(2):
                        for c in range(NCOMP):
                            nc.tensor.matmul(
                                pst[hh][c * 32:(c + 1) * 32, :],
                                kt_sb[hp][hh * 64:(hh + 1) * 64,
                                          TB + c * 32:TB + (c + 1) * 32],
                                qt[c][hh * 64:(hh + 1) * 64, :],
                                start=True, stop=True,
                                skip_group_check=True,
                                tile_position=(hh * 64, c * 32))
                    for hh in range(2):
                        nc.scalar.activation(
                            ex_t[:, hh * 512:(hh + 1) * 512], pst[hh][:],
                            AF.Exp)

                    # ---- cross-component softmax, main part (c on the
                    # free dim -> fused wide DVE ops) ----
                    padd = bc.tile([128, 2, 2 * SL], BF16, tag="padd", bufs=1,
                                   name=f"pa{hp}")
                    nc.vector.tensor_add(out=padd[:], in0=ex_big[:, 0:2, :],
                                         in1=ex_big[:, 2:4, :])
                    ssum = bc.tile([128, 2 * SL], F32, tag="ssum", bufs=1,
                                   name=f"sm{hp}")
                    nc.vector.tensor_add(out=ssum[:], in0=padd[:, 0, :],
                                         in1=padd[:, 1, :])
                    rinv = bc.tile([128, 2 * SL], F32, tag="rinv", bufs=1,
                                   name=f"ri{hp}")
                    nc.vector.reciprocal_approx_fast(out=rinv[:], in_=ssum[:])
                    rinvb = bc.tile([128, 2 * SL], BF16, tag="rinvb", bufs=1,
                                    name=f"rb{hp}")
                    nc.vector.tensor_copy(out=rinvb[:], in_=rinv[:])
                    # normalize in place: ex_big becomes w_big
                    nc.vector.tensor_mul(
                        out=ex_big[:], in0=ex_big[:],
                        in1=rinvb.unsqueeze(1).broadcast_to(
                            [128, NCOMP, 2 * SL]))
                    # ex_t is [128, 4*512] with hh along free; view the AV
                    # slices as [128, SL] via the hh packing
                    pend = (hp, ex_big, ex_t)

                if pend is not None:
                    rinvt_prev = _tail_sum(pend[0], pend[2])
                    _tail_mul_av(pend[0], pend[1], pend[2], rinvt_prev)
                    pend = None

                # ----- phase C: out-proj + bias + residual -----
                # the first four chains borrow the pq/ps PSUM tags, whose
                # banks free early in the last head-pair (after the q
                # copies / exps), so the PE fills the gap while the final
                # deferred softmax+AV runs on the scalar/vector engines
                for fot in range(FT if "C" in phases else 0):
                    if fot < 3:
                        wo_b = wo_pre[fot]
                    else:
                        wo_b = bc.tile([128, FT * 128], BF16, tag="wo",
                                       bufs=3, name=f"wo{fot}")
                        nc.sync.dma_start(
                            out=wo_b.rearrange("p (f o) -> p f o", f=FT),
                            in_=woT_v[:, :, fot * 128:(fot + 1) * 128])
                    for c in range(NCOMP):
                        if fot == 0:
                            tag, nb = ("pq", 2) if c < 2 else ("ps", 3)
                        else:
                            tag, nb = "po", 3
                        po = bcp.tile([128, SL], F32, tag=tag, bufs=nb,
                                      name=f"pc{fot}_{c}")
                        for fi in range(FT):
                            nc.tensor.matmul(
                                po[:], wo_b[:, fi * 128:(fi + 1) * 128],
                                ot_sb[(c, fi)][:],
                                start=(fi == 0), stop=(fi == FT - 1))
                        ob = bc.tile([128, SL], BF16, tag="outsb", bufs=3,
                                     name=f"ob{fot}_{c}")
                        nc.vector.scalar_tensor_tensor(
                            out=ob[:], in0=po[:],
                            scalar=bo_sb[:, fot:fot + 1],
                            in1=xh[c][:, fot * SL:(fot + 1) * SL],
                            op0=ALU.add, op1=ALU.add)
                        # scalar queue is idle during phase C
                        nc.scalar.dma_start(
                            out=outT_v[c][:, fot, :], in_=ob[:])

        repeat = int(os.environ.get("K_REPEAT", "1"))
        for _rep in range(repeat):
            _phases()


_NC_CACHE = {}


def _get_nc():
    if "nc" not in _NC_CACHE:
        nc = bacc.Bacc("TRN2", target_bir_lowering=False)
        with tile.TileContext(nc) as tc:
            _emit(tc)
        nc.compile()
        _NC_CACHE["nc"] = nc
    return _NC_CACHE["nc"]


def kernel(hidden_states, encoder_hidden_states, temperature, Wq, Wk, Wv, Wo,
           bo, pad_length):
    # pad branch contributes zero to the output (zeros projected with no
    # bias give k_pad = v_pad = 0), so pad_length is irrelevant.
    hs = np.ascontiguousarray(np.asarray(hidden_states, dtype=np.float32))
    ehs = np.ascontiguousarray(
        np.asarray(encoder_hidden_states, dtype=np.float32))
    temp = float(np.asarray(temperature).reshape(-1)[0])
    Wq = np.asarray(Wq, dtype=np.float32)
    Wk = np.asarray(Wk, dtype=np.float32)
    Wv = np.asarray(Wv, dtype=np.float32)
    Wo = np.asarray(Wo, dtype=np.float32)
    bo_v = np.asarray(bo, dtype=np.float32).reshape(-1)

    wqT = np.ascontiguousarray((Wq / (temp + EPS)).T).astype(ml_dtypes.bfloat16)
    # wk packed partition-major per 128-col block: wkT[b, p, fi*128+j] =
    # Wk.T[fi*128+p, b*128+j]
    wkT = np.ascontiguousarray(
        Wk.T.reshape(FT, 128, FT, 128).transpose(2, 1, 0, 3).reshape(
            FT, 128, FT * 128)).astype(ml_dtypes.bfloat16)
    wvT = np.ascontiguousarray(Wv.T).astype(ml_dtypes.bfloat16)
    woT = np.ascontiguousarray(Wo.T).astype(ml_dtypes.bfloat16)
    eT_all = np.zeros((D, ECAT), dtype=np.float32)
    for c in range(NCOMP):
        eT_all[:, c * EM:(c + 1) * EM] = ehs[c].T[:, :EM]
        eT_all[:, TB + c * 32:TB + c * 32 + ET] = ehs[c].T[:, EM:E]
    eT_all = eT_all.astype(ml_dtypes.bfloat16)
    bo_t = np.ascontiguousarray(bo_v.reshape(FT, 128).T)

    # msum sums the 4 component groups and broadcasts the sum back to
    # every group: tps[c*32+j, s] = sum_c' ex[c'*32+j, s] for j < 26;
    # pad rows j >= 26 get their own value (=1) so 1/x stays finite
    msum_h = np.zeros((128, 128), dtype=np.float32)
    for c in range(NCOMP):
        for j in range(ET):
            for cp in range(NCOMP):
                msum_h[cp * 32 + j, c * 32 + j] = 1.0
        for j in range(ET, 32):
            msum_h[c * 32 + j, c * 32 + j] = 1.0

    nc = _get_nc()
    in_maps = []
    for i in range(NCORES):
        xT_i = np.ascontiguousarray(
            hs[:, i * SL:(i + 1) * SL, :].transpose(0, 2, 1)).astype(
                ml_dtypes.bfloat16)
        in_maps.append({
            "xTb": xT_i, "eT": eT_all, "wqT": wqT, "wkT": wkT,
            "wvT": wvT, "woT": woT, "bo": bo_t,
            "msum": msum_h.astype(ml_dtypes.bfloat16),
        })

    res = run_bass_kernel_spmd(nc, in_maps, core_ids=list(range(NCORES)))

    out = np.empty((NCOMP, S, D), dtype=np.float32)
    for i in range(NCORES):
        out[:, i * SL:(i + 1) * SL, :] = res.results[i]["outT"].astype(
            np.float32).transpose(0, 2, 1)
    return out


# revision 42
# speedup vs baseline: 1.0078x; 1.0078x over previous
"""Trainium2 Bass kernel for DecomposingAttnProcessor (pad variant).

Math (pad branch contributes exactly zero since pad tokens are zeros
projected with no bias -> k_pad = v_pad = 0):
    q = hs @ Wq.T / (temp + eps)   (scale folded into Wq on host)
    k = ehs @ Wk.T ; v = ehs @ Wv.T
    scores[c,h,s,e] = q . k        (per head, dh=64)
    w = softmax over the 4 components c (dim 0)
    o = w @ v ; out = o @ Wo.T + bo + hs

Sharding: 8 cores, split S=4096 into 512-row blocks; all 4 components of
a block stay on one core (softmax couples them). K/V computed redundantly
per core (encoder seq is only 154).

All matmuls run at N=512 (one full PSUM bank) to amortize the per-matmul
LDWEIGHTS/dispatch tax; the whole 512-row s-block is processed in one
pass (no s-halves).

Encoder layout (host-packed, 640 columns):
    cols [c*128,(c+1)*128) = component c, e in [0,128)   ("main")
    cols [512+c*32, 512+c*32+32) = component c, e in [128,154) zero-padded
    to 32 ("tail") so every matmul operand is 32-aligned on partitions.

Concurrency: small matmuls (tails, score halves) are issued back-to-back
to disjoint 32-strip PE tiles (tile_position) so they run concurrently
instead of serially; the tail softmax sum+broadcast is a single constant
matmul (Msum) instead of two.

DMA: eT/wv/xh/wq/wo stream on the sync queue; wk blocks + wq00 + small
constants go on the otherwise-idle gpsimd queue so the two queues issue
descriptors in parallel at kernel start.  Outputs (bf16) are written from
the scalar queue, which is idle during the out-projection phase.
"""

import numpy as np
import ml_dtypes

import concourse.bass as bass
import concourse.mybir as mybir
import concourse.tile as tile
from concourse import bacc
from concourse.bass_utils import run_bass_kernel_spmd

F32 = mybir.dt.float32
BF16 = mybir.dt.bfloat16
AF = mybir.ActivationFunctionType
ALU = mybir.AluOpType

NCOMP = 4
HEADS = 24
DH = 64
D = 1536
S = 4096
E = 154
EM = 128                  # main e-rows per component
ET = E - EM               # 26 tail e-rows per component
EPS = 1e-8
NCORES = 8
SL = S // NCORES          # 512 s-rows per core (per component)
FT = D // 128             # 12 feature tiles of 128
HP = HEADS // 2           # 12 head-pairs (2 heads = 128 feature rows)
ECAT = 640                # 4*128 main + 4*32 padded tail columns
TB = 4 * EM               # 512: tail block column base


def _emit(tc):
    import os
    phases = os.environ.get("K_PHASES", "ABC")
    blevel = int(os.environ.get("K_BLEVEL", "4"))
    nc = tc.nc

    xTb = nc.declare_dram_parameter("xTb", [NCOMP, D, SL], BF16, isOutput=False)
    eT = nc.declare_dram_parameter("eT", [D, ECAT], BF16, isOutput=False)
    wqT = nc.declare_dram_parameter("wqT", [D, D], BF16, isOutput=False)
    wkT = nc.declare_dram_parameter("wkT", [D, D], BF16, isOutput=False)
    wvT = nc.declare_dram_parameter("wvT", [D, D], BF16, isOutput=False)
    woT = nc.declare_dram_parameter("woT", [D, D], BF16, isOutput=False)
    bo = nc.declare_dram_parameter("bo", [128, FT], F32, isOutput=False)
    msum = nc.declare_dram_parameter("msum", [128, 128], BF16, isOutput=False)
    outT = nc.declare_dram_parameter("outT", [NCOMP, D, SL], BF16, isOutput=True)

    xTb_v = [xTb[c].rearrange("(f p) s -> p f s", p=128) for c in range(NCOMP)]
    eT_v = eT.rearrange("(f p) e -> p f e", p=128)
    wqT_v = wqT.rearrange("(f p) o -> p f o", p=128)
    wkT_v = wkT.rearrange("(f p) o -> p f o", p=128)
    wvT_v = wvT.rearrange("(f p) o -> p f o", p=128)
    woT_v = woT.rearrange("(f p) o -> p f o", p=128)
    outT_v = [outT[c].rearrange("(f p) s -> p f s", p=128) for c in range(NCOMP)]

    with tc.tile_pool(name="persist", bufs=1) as pp:
        # ---------------- persistent tiles ----------------
        kt_sb = [pp.tile([128, ECAT], BF16, tag="kT", bufs=FT, name=f"kt{t}")
                 for t in range(FT)]
        vm_sb = [pp.tile([128, D], BF16, tag="vm", bufs=NCOMP, name=f"vm{c}")
                 for c in range(NCOMP)]
        vt_sb = pp.tile([128, D], BF16, tag="vt", bufs=1, name="vt")
        bo_sb = pp.tile([128, FT], F32, tag="bo", bufs=1, name="bo_sb")
        msum_sb = pp.tile([128, 128], BF16, tag="msum", bufs=1, name="msum_sb")

        # x panels (Q rhs + residual source) and the first wq block; bulk
        # loads go on the sync queue behind eT/wv in priority order
        xh = [pp.tile([128, FT * SL], BF16, tag="xh", bufs=NCOMP,
                      name=f"xh_{c}") for c in range(NCOMP)]
        wq00 = pp.tile([128, FT * 128], BF16, tag="wq00", bufs=1, name="wq00")

        def _load_xh(c):
            nc.sync.dma_start(
                out=xh[c].rearrange("p (f s) -> p f s", f=FT), in_=xTb_v[c])

        def _load_wq00():
            # gpsimd queue: parallel with the sync-queue bulk stream
            nc.gpsimd.dma_start(
                out=wq00.rearrange("p (f o) -> p f o", f=FT),
                in_=wqT_v[:, :, 0:128])

        def _phases():
            # ---------------- phase A: K^T and V ----------------
            if "A" in phases:
              with (
                tc.tile_pool(name="pha", bufs=1) as pa,
                tc.tile_pool(name="pha_psum", bufs=1, space="PSUM") as pap,
              ):
                # wk0 then eT in 4 chunks of 3 feature-tiles on the sync
                # queue: few descriptors (the sync queue issues one per
                # ~650ns) but enough that they spread over parallel DMA
                # engines instead of crawling through one
                wk_all = []
                wk0 = pa.tile([128, FT * 128], BF16, tag="wk",
                              bufs=4, name="wk0")
                nc.sync.dma_start(
                    out=wk0.rearrange("p (f o) -> p f o", f=FT),
                    in_=wkT_v[:, :, 0:128])
                wk_all.append(wk0)
                # per-fi 2-level descriptors: a single flat [128, 1280B]
                # transfer sustains ~250 GB/s while 3-level patterns crawl
                # at ~90 GB/s; the second half goes out on the (idle)
                # scalar queue so the two halves stream in parallel
                et_b = pa.tile([128, FT * ECAT], BF16, tag="eT", bufs=1,
                               name="et_b")
                for fi in range(FT // 2):
                    nc.sync.dma_start(
                        out=et_b[:, fi * ECAT:(fi + 1) * ECAT],
                        in_=eT_v[:, fi])
                for fi in range(FT // 2, FT):
                    nc.scalar.dma_start(
                        out=et_b[:, fi * ECAT:(fi + 1) * ECAT],
                        in_=eT_v[:, fi])
                et = [et_b[:, fi * ECAT:(fi + 1) * ECAT] for fi in range(FT)]

                # remaining wk blocks on the gpsimd queue (parallel issue);
                # small loads right after wk1 so wk2+ don't compete with
                # the critical eT stream for HBM bandwidth
                for fot in range(1, FT):
                    wk_b = pa.tile([128, FT * 128], BF16, tag="wk",
                                   bufs=4, name=f"wk{fot}")
                    nc.gpsimd.dma_start(
                        out=wk_b.rearrange("p (f o) -> p f o", f=FT),
                        in_=wkT_v[:, :, fot * 128:(fot + 1) * 128])
                    wk_all.append(wk_b)
                    if fot == 1:
                        nc.gpsimd.dma_start(out=bo_sb[:], in_=bo[:])
                        nc.gpsimd.dma_start(out=msum_sb[:], in_=msum[:])
                    if fot == 6:
                        _load_wq00()

                # wv + xh stream on sync behind eT, two descriptors each
                wv_all = []
                for fvc in range(3):
                    wv_b = pa.tile([128, FT * 512], BF16, tag="wv", bufs=3,
                                   name=f"wv{fvc}")
                    wv_v3 = wv_b.rearrange("p (f o) -> p f o", f=FT)
                    for ch in range(2):
                        nc.sync.dma_start(
                            out=wv_v3[:, 6 * ch:6 * ch + 6],
                            in_=wvT_v[:, 6 * ch:6 * ch + 6,
                                      fvc * 512:(fvc + 1) * 512])
                    wv_all.append(wv_b)
                for c in range(NCOMP):
                    xh_v3 = xh[c].rearrange("p (f s) -> p f s", f=FT)
                    for ch in range(2):
                        nc.sync.dma_start(
                            out=xh_v3[:, 6 * ch:6 * ch + 6],
                            in_=xTb_v[c][:, 6 * ch:6 * ch + 6])

                # K^T[fo, col] over fi; N split 320+320
                for fot in range(FT):
                    wk_b = wk_all[fot]
                    for nch in range(2):
                        n0 = nch * 320
                        pk = pap.tile([128, 320], F32, tag="pk", bufs=2,
                                      name=f"pk{fot}_{nch}")
                        for fi in range(FT):
                            nc.tensor.matmul(
                                pk[:], wk_b[:, fi * 128:(fi + 1) * 128],
                                et[fi][:, n0:n0 + 320],
                                start=(fi == 0), stop=(fi == FT - 1))
                        nc.scalar.copy(kt_sb[fot][:, n0:n0 + 320], pk[:])

                # V (natural layout [e, dv], bf16) over fi; tails of all 4
                # components go concurrently to disjoint column groups
                for fvc in range(3):
                    wv_b = wv_all[fvc]
                    for c in range(NCOMP):
                        pv = pap.tile([128, 512], F32, tag="pv", bufs=2,
                                      name=f"pv{fvc}_{c}")
                        for fi in range(FT):
                            nc.tensor.matmul(
                                pv[:],
                                et[fi][:, c * EM:(c + 1) * EM],
                                wv_b[:, fi * 512:(fi + 1) * 512],
                                start=(fi == 0), stop=(fi == FT - 1))
                        nc.scalar.copy(
                            vm_sb[c][:, fvc * 512:(fvc + 1) * 512], pv[:])
                    pvt = pap.tile([128, 512], F32, tag="pv", bufs=2,
                                   name=f"pvt{fvc}")
                    for fi in range(FT):
                        for c in range(NCOMP):
                            nc.tensor.matmul(
                                pvt[c * 32:(c + 1) * 32, :],
                                et[fi][:, TB + c * 32:TB + (c + 1) * 32],
                                wv_b[:, fi * 512:(fi + 1) * 512],
                                start=(fi == 0), stop=(fi == FT - 1),
                                skip_group_check=True,
                                tile_position=(0, c * 32))
                    nc.scalar.copy(
                        vt_sb[:, fvc * 512:(fvc + 1) * 512], pvt[:])
            if "A" not in phases:
                nc.gpsimd.dma_start(out=bo_sb[:], in_=bo[:])
                nc.gpsimd.dma_start(out=msum_sb[:], in_=msum[:])
                for c in range(NCOMP):
                    _load_xh(c)
                _load_wq00()

            # ---------------- phase B: Q, scores, softmax, o ----------------
            with tc.tile_pool(name="bc", bufs=1) as bc:
              with tc.tile_pool(name="bcp", bufs=1, space="PSUM") as bcp:
                ot_sb = {}
                pend = None     # (hp, w_big, ex_t, rinvt) awaiting mul+AV

                # pre-issue the first wo loads so phase C starts without a
                # DMA wait (ring slots recycle for fot >= 3)
                wo_pre = []
                for fot in range(3 if "C" in phases else 0):
                    wo_b = bc.tile([128, FT * 128], BF16, tag="wo", bufs=3,
                                   name=f"wo{fot}")
                    nc.sync.dma_start(
                        out=wo_b.rearrange("p (f o) -> p f o", f=FT),
                        in_=woT_v[:, :, fot * 128:(fot + 1) * 128])
                    wo_pre.append(wo_b)

                def _tail_sum(hp, ex_t):
                    # one constant matmul sums the 4 component groups AND
                    # re-broadcasts the sum to every group's partitions:
                    # tps[c*32+j, s] = sum_c' ex_t[c'*32+j, s]
                    rinvt = bc.tile([128, 2 * SL], F32, tag="rinvt", bufs=2,
                                    name=f"rinvt{hp}")
                    for sh in range(2):
                        tps = bcp.tile([128, 512], F32, tag="ps", bufs=3,
                                       name=f"tps{hp}_{sh}")
                        nc.tensor.matmul(tps[:], msum_sb[:],
                                         ex_t[:, sh * 512:(sh + 1) * 512],
                                         start=True, stop=True)
                        nc.vector.reciprocal_approx_fast(
                            out=rinvt[:, sh * 512:(sh + 1) * 512],
                            in_=tps[:])
                    return rinvt

                def _tail_mul_av(hp, w_big, ex_t, rinvt):
                    # normalize the tail in place (on the otherwise-idle
                    # gpsimd engine so the DVE stays off this critical
                    # path), then AV: mains first (col-paired), then all 8
                    # tails concurrently into disjoint 32-strip PE tiles
                    # accumulating on the mains
                    nc.vector.tensor_mul(out=ex_t[:], in0=ex_t[:],
                                         in1=rinvt[:])
                    w_t = ex_t
                    for cp in range(2 if blevel >= 4 else 0):
                        po = {}
                        for c in (2 * cp, 2 * cp + 1):
                            po[c] = bcp.tile([128, SL], F32, tag="po", bufs=3,
                                             name=f"po{hp}_{c}")
                        for c in (2 * cp, 2 * cp + 1):
                            h0 = hp * 2
                            for hh in range(2):
                                nc.tensor.matmul(
                                    po[c][hh * 64:(hh + 1) * 64, :],
                                    vm_sb[c][:, (h0 + hh) * 64:
                                             (h0 + hh + 1) * 64],
                                    w_big[:, c, hh * SL:(hh + 1) * SL],
                                    start=True, stop=False,
                                    skip_group_check=True,
                                    tile_position=(0, hh * 64))
                        for c in (2 * cp, 2 * cp + 1):
                            h0 = hp * 2
                            for hh in range(2):
                                nc.tensor.matmul(
                                    po[c][hh * 64:(hh + 1) * 64, :],
                                    vt_sb[c * 32:c * 32 + ET,
                                          (h0 + hh) * 64:(h0 + hh + 1) * 64],
                                    w_t[c * 32:c * 32 + ET,
                                        hh * SL:(hh + 1) * SL],
                                    start=False, stop=True,
                                    skip_group_check=True,
                                    tile_position=(c * 32, hh * 64))
                        for c in (2 * cp, 2 * cp + 1):
                            ot = bc.tile([128, SL], BF16, tag="oT", bufs=48,
                                         name=f"ot{hp}_{c}")
                            # scalar engine: the DVE queue is busier here
                            nc.scalar.copy(ot[:], po[c][:])
                            ot_sb[(c, hp)] = ot

                for hp in range(HP if "B" in phases else 0):
                    if hp == 0:
                        wq_b = wq00
                    else:
                        wq_b = bc.tile([128, FT * 128], BF16, tag="wq",
                                       bufs=3, name=f"wq{hp}")
                        nc.sync.dma_start(
                            out=wq_b.rearrange("p (f o) -> p f o", f=FT),
                            in_=wqT_v[:, :, hp * 128:(hp + 1) * 128])

                    # Q^T chains, interleaved with the previous hp's
                    # deferred tail-softmax ops so the PE never waits
                    # on the scalar/vector engines
                    qt = []
                    rinvt_prev = None
                    for c in range(NCOMP):
                        pq = bcp.tile([128, SL], F32, tag="pq", bufs=2,
                                      name=f"pq{hp}_{c}")
                        for fi in range(FT):
                            nc.tensor.matmul(
                                pq[:], wq_b[:, fi * 128:(fi + 1) * 128],
                                xh[c][:, fi * SL:(fi + 1) * SL],
                                start=(fi == 0), stop=(fi == FT - 1))
                        q = bc.tile([128, SL], BF16, tag="qT", bufs=8,
                                    name=f"qt{hp}_{c}")
                        nc.vector.tensor_copy(out=q[:], in_=pq[:])
                        qt.append(q)
                        if c == 1 and pend is not None:
                            rinvt_prev = _tail_sum(pend[0], pend[2])
                    if pend is not None:
                        _tail_mul_av(pend[0], pend[1], pend[2], rinvt_prev)
                        pend = None
                    if blevel < 2:
                        continue

                    # scores + exp; main part: c along the free dim of
                    # one big tile; the two head-halves of each component
                    # go back-to-back into disjoint row strips (concurrent)
                    ex_big = bc.tile([128, NCOMP, 2 * SL], BF16, tag="exb",
                                     bufs=2, name=f"exb{hp}")
                    for c in range(NCOMP):
                        ps = {}
                        for hh in range(2):
                            ps[hh] = bcp.tile([128, SL], F32, tag="ps",
                                              bufs=3, name=f"ps{hp}_{c}_{hh}")
                            nc.tensor.matmul(
                                ps[hh][:],
                                kt_sb[hp][hh * 64:(hh + 1) * 64,
                                          c * EM:(c + 1) * EM],
                                qt[c][hh * 64:(hh + 1) * 64, :],
                                start=True, stop=True,
                                skip_group_check=True,
                                tile_position=(hh * 64, 0))
                        for hh in range(2):
                            # heads packed side by side along the free dim
                            nc.scalar.activation(
                                ex_big[:, c, hh * 512:(hh + 1) * 512],
                                ps[hh][:], AF.Exp)
                    # tail scores: all 8 (hh x c) tiles disjoint -> one
                    # concurrent burst across two PSUM banks
                    ex_t = bc.tile([128, 2 * SL], BF16, tag="ext",
                                   bufs=2, name=f"ext{hp}")
                    pst = {}
                    for hh in range(2):
                        pst[hh] = bcp.tile([128, SL], F32, tag="ps", bufs=3,
                                           name=f"pst{hp}_{hh}")
                    for hh in range(2):
                        for c in range(NCOMP):
                            nc.tensor.matmul(
                                pst[hh][c * 32:(c + 1) * 32, :],
                                kt_sb[hp][hh * 64:(hh + 1) * 64,
                                          TB + c * 32:TB + (c + 1) * 32],
                                qt[c][hh * 64:(hh + 1) * 64, :],
                                start=True, stop=True,
                                skip_group_check=True,
                                tile_position=(hh * 64, c * 32))
                    for hh in range(2):
                        nc.scalar.activation(
                            ex_t[:, hh * 512:(hh + 1) * 512], pst[hh][:],
                            AF.Exp)

                    # ---- cross-component softmax, main part (c on the
                    # free dim -> fused wide DVE ops) ----
                    padd = bc.tile([128, 2, 2 * SL], BF16, tag="padd", bufs=1,
                                   name=f"pa{hp}")
                    nc.vector.tensor_add(out=padd[:], in0=ex_big[:, 0:2, :],
                                         in1=ex_big[:, 2:4, :])
                    ssum = bc.tile([128, 2 * SL], F32, tag="ssum", bufs=1,
                                   name=f"sm{hp}")
                    nc.vector.tensor_add(out=ssum[:], in0=padd[:, 0, :],
                                         in1=padd[:, 1, :])
                    rinv = bc.tile([128, 2 * SL], F32, tag="rinv", bufs=1,
                                   name=f"ri{hp}")
                    nc.vector.reciprocal_approx_fast(out=rinv[:], in_=ssum[:])
                    rinvb = bc.tile([128, 2 * SL], BF16, tag="rinvb", bufs=1,
                                    name=f"rb{hp}")
                    nc.vector.tensor_copy(out=rinvb[:], in_=rinv[:])
                    # normalize in place: ex_big becomes w_big
                    nc.vector.tensor_mul(
                        out=ex_big[:], in0=ex_big[:],
                        in1=rinvb.unsqueeze(1).broadcast_to(
                            [128, NCOMP, 2 * SL]))
                    # ex_t is [128, 4*512] with hh along free; view the AV
                    # slices as [128, SL] via the hh packing
                    pend = (hp, ex_big, ex_t)

                if pend is not None:
                    rinvt_prev = _tail_sum(pend[0], pend[2])
                    _tail_mul_av(pend[0], pend[1], pend[2], rinvt_prev)
                    pend = None

                # ----- phase C: out-proj + bias + residual -----
                # the first four chains borrow the pq/ps PSUM tags, whose
                # banks free early in the last head-pair (after the q
                # copies / exps), so the PE fills the gap while the final
                # deferred softmax+AV runs on the scalar/vector engines
                for fot in range(FT if "C" in phases else 0):
                    if fot < 3:
                        wo_b = wo_pre[fot]
                    else:
                        wo_b = bc.tile([128, FT * 128], BF16, tag="wo",
                                       bufs=3, name=f"wo{fot}")
                        nc.sync.dma_start(
                            out=wo_b.rearrange("p (f o) -> p f o", f=FT),
                            in_=woT_v[:, :, fot * 128:(fot + 1) * 128])
                    for c in range(NCOMP):
                        if fot == 0:
                            tag, nb = ("pq", 2) if c < 2 else ("ps", 3)
                        else:
                            tag, nb = "po", 3
                        po = bcp.tile([128, SL], F32, tag=tag, bufs=nb,
                                      name=f"pc{fot}_{c}")
                        for fi in range(FT):
                            nc.tensor.matmul(
                                po[:], wo_b[:, fi * 128:(fi + 1) * 128],
                                ot_sb[(c, fi)][:],
                                start=(fi == 0), stop=(fi == FT - 1))
                        ob = bc.tile([128, SL], BF16, tag="outsb", bufs=3,
                                     name=f"ob{fot}_{c}")
                        nc.vector.scalar_tensor_tensor(
                            out=ob[:], in0=po[:],
                            scalar=bo_sb[:, fot:fot + 1],
                            in1=xh[c][:, fot * SL:(fot + 1) * SL],
                            op0=ALU.add, op1=ALU.add)
                        # scalar queue is idle during phase C
                        nc.scalar.dma_start(
                            out=outT_v[c][:, fot, :], in_=ob[:])

        repeat = int(os.environ.get("K_REPEAT", "1"))
        for _rep in range(repeat):
            _phases()


_NC_CACHE = {}


def _get_nc():
    if "nc" not in _NC_CACHE:
        nc = bacc.Bacc("TRN2", target_bir_lowering=False)
        with tile.TileContext(nc) as tc:
            _emit(tc)
        nc.compile()
        _NC_CACHE["nc"] = nc
    return _NC_CACHE["nc"]


def kernel(hidden_states, encoder_hidden_states, temperature, Wq, Wk, Wv, Wo,
           bo, pad_length):
    # pad branch contributes zero to the output (zeros projected with no
    # bias give k_pad = v_pad = 0), so pad_length is irrelevant.
    hs = np.ascontiguousarray(np.asarray(hidden_states, dtype=np.float32))
    ehs = np.ascontiguousarray(
        np.asarray(encoder_hidden_states, dtype=np.float32))
    temp = float(np.asarray(temperature).reshape(-1)[0])
    Wq = np.asarray(Wq, dtype=np.float32)
    Wk = np.asarray(Wk, dtype=np.float32)
    Wv = np.asarray(Wv, dtype=np.float32)
    Wo = np.asarray(Wo, dtype=np.float32)
    bo_v = np.asarray(bo, dtype=np.float32).reshape(-1)

    wqT = np.ascontiguousarray((Wq / (temp + EPS)).T).astype(ml_dtypes.bfloat16)
    wkT = np.ascontiguousarray(Wk.T).astype(ml_dtypes.bfloat16)
    wvT = np.ascontiguousarray(Wv.T).astype(ml_dtypes.bfloat16)
    woT = np.ascontiguousarray(Wo.T).astype(ml_dtypes.bfloat16)
    eT_all = np.zeros((D, ECAT), dtype=np.float32)
    for c in range(NCOMP):
        eT_all[:, c * EM:(c + 1) * EM] = ehs[c].T[:, :EM]
        eT_all[:, TB + c * 32:TB + c * 32 + ET] = ehs[c].T[:, EM:E]
    eT_all = eT_all.astype(ml_dtypes.bfloat16)
    bo_t = np.ascontiguousarray(bo_v.reshape(FT, 128).T)

    # msum sums the 4 component groups and broadcasts the sum back to
    # every group: tps[c*32+j, s] = sum_c' ex[c'*32+j, s] for j < 26;
    # pad rows j >= 26 get their own value (=1) so 1/x stays finite
    msum_h = np.zeros((128, 128), dtype=np.float32)
    for c in range(NCOMP):
        for j in range(ET):
            for cp in range(NCOMP):
                msum_h[cp * 32 + j, c * 32 + j] = 1.0
        for j in range(ET, 32):
            msum_h[c * 32 + j, c * 32 + j] = 1.0

    nc = _get_nc()
    in_maps = []
    for i in range(NCORES):
        xT_i = np.ascontiguousarray(
            hs[:, i * SL:(i + 1) * SL, :].transpose(0, 2, 1)).astype(
                ml_dtypes.bfloat16)
        in_maps.append({
            "xTb": xT_i, "eT": eT_all, "wqT": wqT, "wkT": wkT,
            "wvT": wvT, "woT": woT, "bo": bo_t,
            "msum": msum_h.astype(ml_dtypes.bfloat16),
        })

    res = run_bass_kernel_spmd(nc, in_maps, core_ids=list(range(NCORES)))

    out = np.empty((NCOMP, S, D), dtype=np.float32)
    for i in range(NCORES):
        out[:, i * SL:(i + 1) * SL, :] = res.results[i]["outT"].astype(
            np.float32).transpose(0, 2, 1)
    return out
